# revision 16
# baseline (speedup 1.0000x reference)
"""Trainium2 Bass kernel for nn_AverageAttention (B=8, L=2048, D=1024).

Math (per batch b):
    avg[t]  = cumsum(x, axis=t)[t] / (t+1)
    g       = concat([x, avg], -1) @ W_gate.T + b_gate        # (L, 2*D)
    out     = sigmoid(g[:, :D]) * x + sigmoid(g[:, D:]) * avg

Strategy: batch-parallel over 8 NeuronCores (one sequence per core), W_gate
replicated. On-chip layout is transposed (feature-on-partition,
token-on-free) so the cumulative sum is a single DVE tensor_tensor_scan per
128-feature chunk. The gating matmul runs in fp8_e4m3 with
perf_mode=DoubleRow (two 128-row contraction chunks per instruction, ~1.4x
bf16 PE throughput); operands are pre-scaled (x*16, avg*16, W*64) to keep
fp8 out of the subnormal range, and the 1/1024 descale plus bias fold into
the sigmoid PSUM-evacuation on the scalar engine. HBM traffic is kept
minimal: fp8 weights ride in (input+forget) pair tiles, invd is bf16, and
both outputs are stored bf16 and upcast on the host (combined rel err
~1.15e-2 vs the 2e-2 gate).
"""

from contextlib import ExitStack

import ml_dtypes
import numpy as np

import concourse.bass as bass
import concourse.bass_utils as bass_utils
import concourse.mybir as mybir
import concourse.tile as tile
from concourse import bacc
from concourse._compat import with_exitstack
from concourse.bass import ts

B, L, D = 8, 2048, 1024
NJ = D // 128        # 8 feature chunks of x / avg (= DoubleRow pairs)
NOB = 2 * D // 128   # 16 output-feature blocks of g
TCW = 512            # matmul moving free-dim (DoubleRow max: 2*512)
NTC = L // TCW       # token chunks

# fp8 pre-scales. S = AX*BW = AA*BW must be uniform across both halves so
# the PSUM accumulation is uniformly scaled; sigmoid evacuation applies 1/S.
AX = 16.0            # x -> fp8 scale
AA = 16.0            # avg -> fp8 scale
BW = 64.0            # W -> fp8 scale
SINV = 1.0 / (AX * BW)

FP32 = mybir.dt.float32
BF16 = mybir.dt.bfloat16
FP8 = mybir.dt.float8e4


@with_exitstack
def _tile_body(ctx: ExitStack, tc: tile.TileContext, reps: int = 1):
    nc = tc.nc

    xT = nc.dram_tensor("xT", (NJ, 128, L), FP32, kind="ExternalInput").ap()
    # wdr[p]: DoubleRow weight tiles for the (input-gate p, forget-gate p)
    # output-block pair, so one DMA fetches both halves of a pair.
    wdr = nc.dram_tensor(
        "wdr", (NJ, 128, 2, NJ, 2, 128), FP8, kind="ExternalInput"
    ).ap()
    invd = nc.dram_tensor("invd", (128, L), BF16, kind="ExternalInput").ap()
    biash = nc.dram_tensor("biash", (128, NOB), FP32, kind="ExternalInput").ap()
    avgT = nc.dram_tensor("avgT", (NJ, 128, L), BF16, kind="ExternalOutput").ap()
    gatT = nc.dram_tensor("gatT", (NJ, 128, L), BF16, kind="ExternalOutput").ap()

    const_pool = ctx.enter_context(tc.tile_pool(name="const", bufs=1))
    x_pool = ctx.enter_context(tc.tile_pool(name="x", bufs=NJ))
    at_pool = ctx.enter_context(tc.tile_pool(name="at", bufs=2))
    abf_pool = ctx.enter_context(tc.tile_pool(name="abf", bufs=NJ))
    cat_pool = ctx.enter_context(tc.tile_pool(name="cat", bufs=NJ))
    w_pool = ctx.enter_context(tc.tile_pool(name="w", bufs=3))
    sig_pool = ctx.enter_context(tc.tile_pool(name="sig", bufs=3))
    stl_pool = ctx.enter_context(tc.tile_pool(name="stl", bufs=4))
    gtb_pool = ctx.enter_context(tc.tile_pool(name="gtb", bufs=2))
    psum_pool = ctx.enter_context(tc.tile_pool(name="psum", bufs=2, space="PSUM"))

    invd_sb = const_pool.tile([128, L], BF16, tag="invd")
    bias_sb = const_pool.tile([128, NOB], FP32, tag="bias")

    for _rep in range(reps):
        # cat[j][:, 0, :] = fp8(AX*x_j); cat[j][:, 1, :] = fp8(AA*avg_j).
        cats = [
            cat_pool.tile([128, 2, L], FP8, tag="cat", name=f"cat{j}")
            for j in range(NJ)
        ]
        xts = [
            x_pool.tile([128, L], FP32, tag="xt", name=f"xt{j}") for j in range(NJ)
        ]
        abfs = [
            abf_pool.tile([128, L], BF16, tag="abf", name=f"abf{j}")
            for j in range(NJ)
        ]

        # DMA head ordering (FIFO per ring): the first weight pair leads the
        # sync ring (the very first matmul needs it and it depends on
        # nothing), then the x chunks stream; constants ride the scalar ring.
        def load_w(p):
            wt = w_pool.tile([128, 2, NJ, 2, 128], FP8, name=f"wt{p}", tag="wt")
            nc.sync.dma_start(wt[:], wdr[p])
            return wt

        w_tiles = {0: load_w(0)}
        if _rep == 0:
            nc.scalar.dma_start(invd_sb[:], invd[:])
            nc.scalar.dma_start(bias_sb[:], biash[:])

        # Phase 1: per chunk j: load x, cast fp8 x (gpsimd), cumsum scan +
        # 1/(t+1) scale (DVE), cast fp8 avg (scalar) and bf16 avg (gpsimd;
        # feeds both the avgT store and the final combine). Chunk 0 runs in
        # two token pieces (piece-chained scan carry) so cat[0] unblocks the
        # PE's first accumulation group earlier.
        PIECES = {0: 2}
        for j in range(NJ):
            np_j = PIECES.get(j, 1)
            pw = L // np_j
            for p in range(np_j):
                nc.sync.dma_start(xts[j][:, ts(p, pw)], xT[j][:, ts(p, pw)])
        at_tiles = []
        for j in range(NJ):
            xt = xts[j]
            ct = sig_pool.tile([128, L], FP32, tag="st", name=f"ct{j}")
            at = at_pool.tile([128, L], FP32)
            np_j = PIECES.get(j, 1)
            pw = L // np_j
            for p in range(np_j):
                s = ts(p, pw)
                nc.gpsimd.tensor_scalar_mul(cats[j][:, 0, s], xt[:, s], AX)
                nc.vector.tensor_tensor_scan(
                    ct[:, s],
                    xt[:, s],
                    xt[:, s],
                    0.0 if p == 0 else ct[:, p * pw - 1 : p * pw],
                    mybir.AluOpType.add,
                    mybir.AluOpType.bypass,
                )
                nc.vector.tensor_mul(at[:, s], ct[:, s], invd_sb[:, s])
                nc.scalar.activation(
                    cats[j][:, 1, s],
                    at[:, s],
                    mybir.ActivationFunctionType.Copy,
                    scale=AA,
                )
            # The bf16 avg copy for chunk j-1 trails by one chunk so the
            # Pool FIFO never stalls the next chunk's fp8 x-cast on it.
            at_tiles.append(at)
            if j > 0:
                nc.gpsimd.tensor_copy(abfs[j - 1][:], at_tiles[j - 1][:])
                nc.scalar.dma_start(avgT[j - 1], abfs[j - 1][:])
        nc.gpsimd.tensor_copy(abfs[NJ - 1][:], at_tiles[NJ - 1][:])
        nc.scalar.dma_start(avgT[NJ - 1], abfs[NJ - 1][:])

        # Phase 2: g^T blocks via fp8 DoubleRow matmul, weight-stationary
        # across the 4 token-chunk PSUM groups. Output blocks run in
        # (input_gate p, forget_gate p) pairs so each chunk's gate combine
        # runs (and its SBUF frees) as early as possible.
        for p in range(NJ - 1):
            if p + 1 < NJ:
                w_tiles[p + 1] = load_w(p + 1)
            wt = w_tiles.pop(p)
            st_i = sig_pool.tile([128, L], FP32, name=f"sti{p}", tag="st")
            st_f = sig_pool.tile([128, L], FP32, name=f"stf{p}", tag="st")
            # Halves run sequentially: half 0's group closes mid-pair, so its
            # sigmoid (and the PSUM-recycle it gates) hides completely under
            # half 1's matmul stream.
            for half, st in ((0, st_i), (1, st_f)):
                psb = psum_pool.tile([128, L], FP32, name="psb", tag="psb")
                for j in range(NJ):
                    for tcx in range(NTC):
                        nc.tensor.matmul(
                            psb[:, ts(tcx, TCW)],
                            wt[:, half, j, :, :],
                            cats[j][:, :, ts(tcx, TCW)],
                            start=(j == 0),
                            stop=(j == NJ - 1),
                            perf_mode=mybir.MatmulPerfMode.DoubleRow,
                        )
                ob = p + NJ * half
                nc.scalar.activation(
                    st[:],
                    psb[:],
                    mybir.ActivationFunctionType.Sigmoid,
                    bias=bias_sb[:, ob : ob + 1],
                    scale=SINV,
                )
            # Combine and store (bf16). Pairs 0..NJ-3 run the forget-term
            # multiply on the otherwise-idle Pool engine; the DVE keeps the
            # input-term mul + add full-width (per-op DVE drain overhead is
            # large on HW).
            gtb = gtb_pool.tile([128, L], BF16, name=f"gtb{p}", tag="gtb")
            if p < NJ - 2:
                nc.gpsimd.tensor_mul(st_f[:], st_f[:], abfs[p][:])
            else:
                nc.vector.tensor_mul(st_f[:], st_f[:], abfs[p][:])
            nc.vector.tensor_mul(st_i[:], st_i[:], xts[p][:])
            nc.vector.tensor_add(gtb[:], st_i[:], st_f[:])
            nc.sync.dma_start(gatT[p], gtb[:])

        # Last pair runs token-chunk-outer so each PSUM group closes early
        # and its sigmoid + combine + store pipeline behind the remaining
        # matmuls, shortening the serial tail after the final matmul.
        p = NJ - 1
        wt = w_tiles.pop(p)
        gtb = gtb_pool.tile([128, L], BF16, name="gtb_last", tag="gtb")
        psb_last = {
            half: psum_pool.tile([128, L], FP32, name=f"psbl{half}", tag="psb")
            for half in (0, 1)
        }
        for tcx in range(NTC):
            s = ts(tcx, TCW)
            stp = {}
            for half in (0, 1):
                ps = psb_last[half]
                for jj in range(NJ):
                    nc.tensor.matmul(
                        ps[:, s],
                        wt[:, half, jj, :, :],
                        cats[jj][:, :, s],
                        start=(jj == 0),
                        stop=(jj == NJ - 1),
                        perf_mode=mybir.MatmulPerfMode.DoubleRow,
                    )
                ob = p + NJ * half
                st = stl_pool.tile([128, TCW], FP32, name=f"stl{half}_{tcx}", tag="stl")
                nc.scalar.activation(
                    st[:],
                    ps[:, s],
                    mybir.ActivationFunctionType.Sigmoid,
                    bias=bias_sb[:, ob : ob + 1],
                    scale=SINV,
                )
                stp[half] = st
            nc.vector.tensor_mul(stp[1][:], stp[1][:], abfs[p][:, s])
            nc.vector.tensor_mul(stp[0][:], stp[0][:], xts[p][:, s])
            nc.vector.tensor_add(gtb[:, s], stp[0][:], stp[1][:])
            nc.sync.dma_start(gatT[p][:, s], gtb[:, s])


_CACHE: dict = {}


def build_nc(reps: int | None = None):
    import os as _os

    if reps is None:
        reps = int(_os.environ.get("KREPS", "1"))
    key = ("nc", reps)
    if key not in _CACHE:
        nc = bacc.Bacc(
            "TRN2",
            target_bir_lowering=False,
            debug=False,
            enable_asserts=True,
            num_devices=B,
        )
        with tile.TileContext(nc) as t:
            _tile_body(t, reps=reps)
        nc.compile()
        _CACHE[key] = nc
    return _CACHE[key]


def prep_shared(W_gate: np.ndarray, b_gate: np.ndarray):
    # wdr[p, k, half, j, dr, m] = BW * W_gate[128*(half*NJ+p) + m,
    #                                         dr*1024 + 128*j + k]
    w = (
        (W_gate.astype(np.float32) * BW)
        .reshape(2, NJ, 128, 2, NJ, 128)      # [half, p, m, dr, j, k]
        .transpose(1, 5, 0, 4, 3, 2)          # [p, k, half, j, dr, m]
    )
    wdr = np.ascontiguousarray(w).astype(ml_dtypes.float8_e4m3)
    invd = np.ascontiguousarray(
        np.broadcast_to(
            1.0 / np.arange(1, L + 1, dtype=np.float32)[None, :], (128, L)
        )
    ).astype(ml_dtypes.bfloat16)
    biash = np.ascontiguousarray(b_gate.astype(np.float32).reshape(NOB, 128).T)
    return wdr, invd, biash


def make_in_maps(inputs: np.ndarray, W_gate: np.ndarray, b_gate: np.ndarray):
    wdr, invd, biash = prep_shared(W_gate, b_gate)
    in_maps = []
    for c in range(B):
        xT_c = np.ascontiguousarray(inputs[c].T).reshape(NJ, 128, L)
        in_maps.append({"xT": xT_c, "wdr": wdr, "invd": invd, "biash": biash})
    return in_maps


def kernel(inputs: np.ndarray, W_gate: np.ndarray, b_gate: np.ndarray, **run_kwargs):
    inputs = np.asarray(inputs, dtype=np.float32)
    W_gate = np.asarray(W_gate, dtype=np.float32)
    b_gate = np.asarray(b_gate, dtype=np.float32)
    assert inputs.shape == (B, L, D)

    in_maps = make_in_maps(inputs, W_gate, b_gate)
    nc = build_nc()
    res = bass_utils.run_bass_kernel_spmd(
        nc, in_maps, core_ids=list(range(B)), **run_kwargs
    )

    gating = np.empty((B, L, D), dtype=np.float32)
    average = np.empty((B, L, D), dtype=np.float32)
    for c in range(B):
        gating[c] = (
            res.results[c]["gatT"].astype(np.float32).reshape(D, L).T
        )
        average[c] = (
            res.results[c]["avgT"].astype(np.float32).reshape(D, L).T
        )
    if run_kwargs:
        _CACHE["last_results"] = res
    return gating, average


# revision 19
# speedup vs baseline: 1.0263x; 1.0263x over previous
"""Trainium2 Bass kernel for nn_AverageAttention (B=8, L=2048, D=1024).

Math (per batch b):
    avg[t]  = cumsum(x, axis=t)[t] / (t+1)
    g       = concat([x, avg], -1) @ W_gate.T + b_gate        # (L, 2*D)
    out     = sigmoid(g[:, :D]) * x + sigmoid(g[:, D:]) * avg

Strategy: batch-parallel over 8 NeuronCores (one sequence per core), W_gate
replicated. On-chip layout is transposed (feature-on-partition,
token-on-free) so the cumulative sum is a single DVE tensor_tensor_scan per
128-feature chunk. The gating matmul runs in fp8_e4m3 with
perf_mode=DoubleRow (two 128-row contraction chunks per instruction, ~1.4x
bf16 PE throughput); operands are pre-scaled (x*16, avg*16, W*64) to keep
fp8 out of the subnormal range, and the 1/1024 descale plus bias fold into
the sigmoid PSUM-evacuation on the scalar engine. HBM traffic is kept
minimal: x ships bf16 (halves the phase-1 load window), fp8 weights ride
in (input+forget) pair tiles, invd is bf16, and both outputs are stored
bf16 and upcast on the host (combined rel err ~1.17e-2 vs the 2e-2 gate).
"""

from contextlib import ExitStack

import ml_dtypes
import numpy as np

import concourse.bass as bass
import concourse.bass_utils as bass_utils
import concourse.mybir as mybir
import concourse.tile as tile
from concourse import bacc
from concourse._compat import with_exitstack
from concourse.bass import ts

B, L, D = 8, 2048, 1024
NJ = D // 128        # 8 feature chunks of x / avg (= DoubleRow pairs)
NOB = 2 * D // 128   # 16 output-feature blocks of g
TCW = 512            # matmul moving free-dim (DoubleRow max: 2*512)
NTC = L // TCW       # token chunks

# fp8 pre-scales. S = AX*BW = AA*BW must be uniform across both halves so
# the PSUM accumulation is uniformly scaled; sigmoid evacuation applies 1/S.
AX = 16.0            # x -> fp8 scale
AA = 16.0            # avg -> fp8 scale
BW = 64.0            # W -> fp8 scale
SINV = 1.0 / (AX * BW)

FP32 = mybir.dt.float32
BF16 = mybir.dt.bfloat16
FP8 = mybir.dt.float8e4


@with_exitstack
def _tile_body(ctx: ExitStack, tc: tile.TileContext, reps: int = 1):
    nc = tc.nc

    xT = nc.dram_tensor("xT", (NJ, 128, L), BF16, kind="ExternalInput").ap()
    # wdr[p]: DoubleRow weight tiles for the (input-gate p, forget-gate p)
    # output-block pair, so one DMA fetches both halves of a pair.
    wdr = nc.dram_tensor(
        "wdr", (NJ, 128, 2, NJ, 2, 128), FP8, kind="ExternalInput"
    ).ap()
    invd = nc.dram_tensor("invd", (128, L), BF16, kind="ExternalInput").ap()
    biash = nc.dram_tensor("biash", (128, NOB), FP32, kind="ExternalInput").ap()
    avgT = nc.dram_tensor("avgT", (NJ, 128, L), BF16, kind="ExternalOutput").ap()
    gatT = nc.dram_tensor("gatT", (NJ, 128, L), BF16, kind="ExternalOutput").ap()

    const_pool = ctx.enter_context(tc.tile_pool(name="const", bufs=1))
    x_pool = ctx.enter_context(tc.tile_pool(name="x", bufs=NJ))
    at_pool = ctx.enter_context(tc.tile_pool(name="at", bufs=2))
    abf_pool = ctx.enter_context(tc.tile_pool(name="abf", bufs=NJ))
    cat_pool = ctx.enter_context(tc.tile_pool(name="cat", bufs=NJ))
    w_pool = ctx.enter_context(tc.tile_pool(name="w", bufs=3))
    sig_pool = ctx.enter_context(tc.tile_pool(name="sig", bufs=3))
    stl_pool = ctx.enter_context(tc.tile_pool(name="stl", bufs=4))
    gtb_pool = ctx.enter_context(tc.tile_pool(name="gtb", bufs=2))
    psum_pool = ctx.enter_context(tc.tile_pool(name="psum", bufs=2, space="PSUM"))

    invd_sb = const_pool.tile([128, L], BF16, tag="invd")
    bias_sb = const_pool.tile([128, NOB], FP32, tag="bias")

    for _rep in range(reps):
        # cat[j][:, 0, :] = fp8(AX*x_j); cat[j][:, 1, :] = fp8(AA*avg_j).
        cats = [
            cat_pool.tile([128, 2, L], FP8, tag="cat", name=f"cat{j}")
            for j in range(NJ)
        ]
        xts = [
            x_pool.tile([128, L], BF16, tag="xt", name=f"xt{j}") for j in range(NJ)
        ]
        abfs = [
            abf_pool.tile([128, L], BF16, tag="abf", name=f"abf{j}")
            for j in range(NJ)
        ]

        # DMA head ordering (FIFO per ring): the first weight pair leads the
        # sync ring (the very first matmul needs it and it depends on
        # nothing), then the x chunks stream; constants ride the scalar ring.
        def load_w(p):
            wt = w_pool.tile([128, 2, NJ, 2, 128], FP8, name=f"wt{p}", tag="wt")
            nc.sync.dma_start(wt[:], wdr[p])
            return wt

        w_tiles = {0: load_w(0)}
        if _rep == 0:
            nc.scalar.dma_start(invd_sb[:], invd[:])
            nc.scalar.dma_start(bias_sb[:], biash[:])

        # Phase 1: per chunk j: load x, cast fp8 x (gpsimd), cumsum scan +
        # 1/(t+1) scale (DVE), cast fp8 avg (scalar) and bf16 avg (gpsimd;
        # feeds both the avgT store and the final combine). Chunk 0 runs in
        # two token pieces (piece-chained scan carry) so cat[0] unblocks the
        # PE's first accumulation group earlier.
        PIECES = {0: 2}
        for j in range(NJ):
            np_j = PIECES.get(j, 1)
            pw = L // np_j
            for p in range(np_j):
                nc.sync.dma_start(xts[j][:, ts(p, pw)], xT[j][:, ts(p, pw)])
        at_tiles = []
        for j in range(NJ):
            xt = xts[j]
            ct = sig_pool.tile([128, L], FP32, tag="st", name=f"ct{j}")
            at = at_pool.tile([128, L], FP32)
            np_j = PIECES.get(j, 1)
            pw = L // np_j
            for p in range(np_j):
                s = ts(p, pw)
                nc.gpsimd.tensor_scalar_mul(cats[j][:, 0, s], xt[:, s], AX)
                nc.vector.tensor_tensor_scan(
                    ct[:, s],
                    xt[:, s],
                    xt[:, s],
                    0.0 if p == 0 else ct[:, p * pw - 1 : p * pw],
                    mybir.AluOpType.add,
                    mybir.AluOpType.bypass,
                )
                nc.vector.tensor_mul(at[:, s], ct[:, s], invd_sb[:, s])
                nc.scalar.activation(
                    cats[j][:, 1, s],
                    at[:, s],
                    mybir.ActivationFunctionType.Copy,
                    scale=AA,
                )
            # The bf16 avg copy for chunk j-1 trails by one chunk so the
            # Pool FIFO never stalls the next chunk's fp8 x-cast on it.
            at_tiles.append(at)
            if j > 0:
                nc.gpsimd.tensor_copy(abfs[j - 1][:], at_tiles[j - 1][:])
                nc.scalar.dma_start(avgT[j - 1], abfs[j - 1][:])
        nc.gpsimd.tensor_copy(abfs[NJ - 1][:], at_tiles[NJ - 1][:])
        nc.scalar.dma_start(avgT[NJ - 1], abfs[NJ - 1][:])

        # Phase 2: g^T blocks via fp8 DoubleRow matmul, weight-stationary
        # across the 4 token-chunk PSUM groups. Output blocks run in
        # (input_gate p, forget_gate p) pairs so each chunk's gate combine
        # runs (and its SBUF frees) as early as possible.
        for p in range(NJ - 1):
            if p + 1 < NJ:
                w_tiles[p + 1] = load_w(p + 1)
            wt = w_tiles.pop(p)
            st_i = sig_pool.tile([128, L], FP32, name=f"sti{p}", tag="st")
            st_f = sig_pool.tile([128, L], FP32, name=f"stf{p}", tag="st")
            # Halves run sequentially: half 0's group closes mid-pair, so its
            # sigmoid (and the PSUM-recycle it gates) hides completely under
            # half 1's matmul stream.
            for half, st in ((0, st_i), (1, st_f)):
                psb = psum_pool.tile([128, L], FP32, name="psb", tag="psb")
                for j in range(NJ):
                    for tcx in range(NTC):
                        nc.tensor.matmul(
                            psb[:, ts(tcx, TCW)],
                            wt[:, half, j, :, :],
                            cats[j][:, :, ts(tcx, TCW)],
                            start=(j == 0),
                            stop=(j == NJ - 1),
                            perf_mode=mybir.MatmulPerfMode.DoubleRow,
                        )
                ob = p + NJ * half
                nc.scalar.activation(
                    st[:],
                    psb[:],
                    mybir.ActivationFunctionType.Sigmoid,
                    bias=bias_sb[:, ob : ob + 1],
                    scale=SINV,
                )
            # Combine and store (bf16). Pairs 0..NJ-3 run the forget-term
            # multiply on the otherwise-idle Pool engine; the DVE keeps the
            # input-term mul + add full-width (per-op DVE drain overhead is
            # large on HW).
            gtb = gtb_pool.tile([128, L], BF16, name=f"gtb{p}", tag="gtb")
            if p < NJ - 2:
                nc.gpsimd.tensor_mul(st_f[:], st_f[:], abfs[p][:])
            else:
                nc.vector.tensor_mul(st_f[:], st_f[:], abfs[p][:])
            nc.vector.tensor_mul(st_i[:], st_i[:], xts[p][:])
            nc.vector.tensor_add(gtb[:], st_i[:], st_f[:])
            nc.sync.dma_start(gatT[p], gtb[:])

        # Last pair runs token-chunk-outer so each PSUM group closes early
        # and its sigmoid + combine + store pipeline behind the remaining
        # matmuls, shortening the serial tail after the final matmul.
        p = NJ - 1
        wt = w_tiles.pop(p)
        gtb = gtb_pool.tile([128, L], BF16, name="gtb_last", tag="gtb")
        psb_last = {
            half: psum_pool.tile([128, L], FP32, name=f"psbl{half}", tag="psb")
            for half in (0, 1)
        }
        for tcx in range(NTC):
            s = ts(tcx, TCW)
            stp = {}
            for half in (0, 1):
                ps = psb_last[half]
                for jj in range(NJ):
                    nc.tensor.matmul(
                        ps[:, s],
                        wt[:, half, jj, :, :],
                        cats[jj][:, :, s],
                        start=(jj == 0),
                        stop=(jj == NJ - 1),
                        perf_mode=mybir.MatmulPerfMode.DoubleRow,
                    )
                ob = p + NJ * half
                st = stl_pool.tile([128, TCW], FP32, name=f"stl{half}_{tcx}", tag="stl")
                nc.scalar.activation(
                    st[:],
                    ps[:, s],
                    mybir.ActivationFunctionType.Sigmoid,
                    bias=bias_sb[:, ob : ob + 1],
                    scale=SINV,
                )
                stp[half] = st
            nc.vector.tensor_mul(stp[1][:], stp[1][:], abfs[p][:, s])
            nc.vector.tensor_mul(stp[0][:], stp[0][:], xts[p][:, s])
            nc.vector.tensor_add(gtb[:, s], stp[0][:], stp[1][:])
            nc.sync.dma_start(gatT[p][:, s], gtb[:, s])


_CACHE: dict = {}


def build_nc(reps: int | None = None):
    import os as _os

    if reps is None:
        reps = int(_os.environ.get("KREPS", "1"))
    key = ("nc", reps)
    if key not in _CACHE:
        nc = bacc.Bacc(
            "TRN2",
            target_bir_lowering=False,
            debug=False,
            enable_asserts=True,
            num_devices=B,
        )
        with tile.TileContext(nc) as t:
            _tile_body(t, reps=reps)
        nc.compile()
        _CACHE[key] = nc
    return _CACHE[key]


def prep_shared(W_gate: np.ndarray, b_gate: np.ndarray):
    # wdr[p, k, half, j, dr, m] = BW * W_gate[128*(half*NJ+p) + m,
    #                                         dr*1024 + 128*j + k]
    w = (
        (W_gate.astype(np.float32) * BW)
        .reshape(2, NJ, 128, 2, NJ, 128)      # [half, p, m, dr, j, k]
        .transpose(1, 5, 0, 4, 3, 2)          # [p, k, half, j, dr, m]
    )
    wdr = np.ascontiguousarray(w).astype(ml_dtypes.float8_e4m3)
    invd = np.ascontiguousarray(
        np.broadcast_to(
            1.0 / np.arange(1, L + 1, dtype=np.float32)[None, :], (128, L)
        )
    ).astype(ml_dtypes.bfloat16)
    biash = np.ascontiguousarray(b_gate.astype(np.float32).reshape(NOB, 128).T)
    return wdr, invd, biash


def make_in_maps(inputs: np.ndarray, W_gate: np.ndarray, b_gate: np.ndarray):
    wdr, invd, biash = prep_shared(W_gate, b_gate)
    in_maps = []
    for c in range(B):
        # x ships bf16: the matmul path quantizes to fp8 anyway, the cumsum
        # scan keeps fp32 state regardless of input dtype, and the combine's
        # bf16-x error (~0.2%) is far inside the 2e-2 budget. Halves the
        # phase-1 HBM window (8 -> 4 MiB).
        xT_c = (
            np.ascontiguousarray(inputs[c].T)
            .reshape(NJ, 128, L)
            .astype(ml_dtypes.bfloat16)
        )
        in_maps.append({"xT": xT_c, "wdr": wdr, "invd": invd, "biash": biash})
    return in_maps


def kernel(inputs: np.ndarray, W_gate: np.ndarray, b_gate: np.ndarray, **run_kwargs):
    inputs = np.asarray(inputs, dtype=np.float32)
    W_gate = np.asarray(W_gate, dtype=np.float32)
    b_gate = np.asarray(b_gate, dtype=np.float32)
    assert inputs.shape == (B, L, D)

    in_maps = make_in_maps(inputs, W_gate, b_gate)
    nc = build_nc()
    res = bass_utils.run_bass_kernel_spmd(
        nc, in_maps, core_ids=list(range(B)), **run_kwargs
    )

    gating = np.empty((B, L, D), dtype=np.float32)
    average = np.empty((B, L, D), dtype=np.float32)
    for c in range(B):
        gating[c] = (
            res.results[c]["gatT"].astype(np.float32).reshape(D, L).T
        )
        average[c] = (
            res.results[c]["avgT"].astype(np.float32).reshape(D, L).T
        )
    if run_kwargs:
        _CACHE["last_results"] = res
    return gating, average


# revision 20
# speedup vs baseline: 1.0291x; 1.0028x over previous
"""Trainium2 Bass kernel for nn_AverageAttention (B=8, L=2048, D=1024).

Math (per batch b):
    avg[t]  = cumsum(x, axis=t)[t] / (t+1)
    g       = concat([x, avg], -1) @ W_gate.T + b_gate        # (L, 2*D)
    out     = sigmoid(g[:, :D]) * x + sigmoid(g[:, D:]) * avg

Strategy: batch-parallel over 8 NeuronCores (one sequence per core), W_gate
replicated. On-chip layout is transposed (feature-on-partition,
token-on-free) so the cumulative sum is a single DVE tensor_tensor_scan per
128-feature chunk. The gating matmul runs in fp8_e4m3 with
perf_mode=DoubleRow (two 128-row contraction chunks per instruction, ~1.4x
bf16 PE throughput); operands are pre-scaled (x*16, avg*16, W*64) to keep
fp8 out of the subnormal range, and the 1/1024 descale plus bias fold into
the sigmoid PSUM-evacuation on the scalar engine. HBM traffic is kept
minimal: x ships bf16 (halves the phase-1 load window), fp8 weights ride
in (input+forget) pair tiles, invd is bf16, and both outputs are stored
bf16 and upcast on the host (combined rel err ~1.17e-2 vs the 2e-2 gate).
"""

from contextlib import ExitStack

import ml_dtypes
import numpy as np

import concourse.bass as bass
import concourse.bass_utils as bass_utils
import concourse.mybir as mybir
import concourse.tile as tile
from concourse import bacc
from concourse._compat import with_exitstack
from concourse.bass import ts

B, L, D = 8, 2048, 1024
NJ = D // 128        # 8 feature chunks of x / avg (= DoubleRow pairs)
NOB = 2 * D // 128   # 16 output-feature blocks of g
TCW = 512            # matmul moving free-dim (DoubleRow max: 2*512)
NTC = L // TCW       # token chunks

# fp8 pre-scales. S = AX*BW = AA*BW must be uniform across both halves so
# the PSUM accumulation is uniformly scaled; sigmoid evacuation applies 1/S.
AX = 16.0            # x -> fp8 scale
AA = 16.0            # avg -> fp8 scale
BW = 64.0            # W -> fp8 scale
SINV = 1.0 / (AX * BW)

FP32 = mybir.dt.float32
BF16 = mybir.dt.bfloat16
FP8 = mybir.dt.float8e4


@with_exitstack
def _tile_body(ctx: ExitStack, tc: tile.TileContext, reps: int = 1):
    nc = tc.nc

    xT = nc.dram_tensor("xT", (NJ, 128, L), BF16, kind="ExternalInput").ap()
    # wdr[p]: DoubleRow weight tiles for the (input-gate p, forget-gate p)
    # output-block pair, so one DMA fetches both halves of a pair.
    wdr = nc.dram_tensor(
        "wdr", (NJ, 128, 2, NJ, 2, 128), FP8, kind="ExternalInput"
    ).ap()
    invd = nc.dram_tensor("invd", (128, L), BF16, kind="ExternalInput").ap()
    biash = nc.dram_tensor("biash", (128, NOB), FP32, kind="ExternalInput").ap()
    avgT = nc.dram_tensor("avgT", (NJ, 128, L), BF16, kind="ExternalOutput").ap()
    gatT = nc.dram_tensor("gatT", (NJ, 128, L), BF16, kind="ExternalOutput").ap()

    const_pool = ctx.enter_context(tc.tile_pool(name="const", bufs=1))
    x_pool = ctx.enter_context(tc.tile_pool(name="x", bufs=NJ))
    at_pool = ctx.enter_context(tc.tile_pool(name="at", bufs=2))
    abf_pool = ctx.enter_context(tc.tile_pool(name="abf", bufs=NJ))
    cat_pool = ctx.enter_context(tc.tile_pool(name="cat", bufs=NJ))
    w_pool = ctx.enter_context(tc.tile_pool(name="w", bufs=3))
    sig_pool = ctx.enter_context(tc.tile_pool(name="sig", bufs=3))
    stl_pool = ctx.enter_context(tc.tile_pool(name="stl", bufs=4))
    gtb_pool = ctx.enter_context(tc.tile_pool(name="gtb", bufs=2))
    psum_pool = ctx.enter_context(tc.tile_pool(name="psum", bufs=2, space="PSUM"))

    invd_sb = const_pool.tile([128, L], BF16, tag="invd")
    bias_sb = const_pool.tile([128, NOB], FP32, tag="bias")

    for _rep in range(reps):
        # cat[j][:, 0, :] = fp8(AX*x_j); cat[j][:, 1, :] = fp8(AA*avg_j).
        cats = [
            cat_pool.tile([128, 2, L], FP8, tag="cat", name=f"cat{j}")
            for j in range(NJ)
        ]
        xts = [
            x_pool.tile([128, L], BF16, tag="xt", name=f"xt{j}") for j in range(NJ)
        ]
        abfs = [
            abf_pool.tile([128, L], BF16, tag="abf", name=f"abf{j}")
            for j in range(NJ)
        ]

        # DMA head ordering (FIFO per ring): the first weight pair leads the
        # sync ring (the very first matmul needs it and it depends on
        # nothing), then the x chunks stream; constants ride the scalar ring.
        def load_w(p):
            wt = w_pool.tile([128, 2, NJ, 2, 128], FP8, name=f"wt{p}", tag="wt")
            nc.sync.dma_start(wt[:], wdr[p])
            return wt

        w_tiles = {0: load_w(0)}
        if _rep == 0:
            nc.scalar.dma_start(invd_sb[:], invd[:])
            nc.scalar.dma_start(bias_sb[:], biash[:])

        # Phase 1: per chunk j: load x, cast fp8 x (gpsimd), cumsum scan +
        # 1/(t+1) scale (DVE), cast fp8 avg (scalar) and bf16 avg (gpsimd;
        # feeds both the avgT store and the final combine). Chunk 0 runs in
        # two token pieces (piece-chained scan carry) so cat[0] unblocks the
        # PE's first accumulation group earlier.
        PIECES = {0: 2}
        for j in range(NJ):
            np_j = PIECES.get(j, 1)
            pw = L // np_j
            for p in range(np_j):
                nc.sync.dma_start(xts[j][:, ts(p, pw)], xT[j][:, ts(p, pw)])
        at_tiles = []
        for j in range(NJ):
            xt = xts[j]
            ct = sig_pool.tile([128, L], FP32, tag="st", name=f"ct{j}")
            at = at_pool.tile([128, L], FP32)
            np_j = PIECES.get(j, 1)
            pw = L // np_j
            for p in range(np_j):
                s = ts(p, pw)
                nc.gpsimd.tensor_scalar_mul(cats[j][:, 0, s], xt[:, s], AX)
                nc.vector.tensor_tensor_scan(
                    ct[:, s],
                    xt[:, s],
                    xt[:, s],
                    0.0 if p == 0 else ct[:, p * pw - 1 : p * pw],
                    mybir.AluOpType.add,
                    mybir.AluOpType.bypass,
                )
                nc.vector.tensor_mul(at[:, s], ct[:, s], invd_sb[:, s])
                nc.scalar.activation(
                    cats[j][:, 1, s],
                    at[:, s],
                    mybir.ActivationFunctionType.Copy,
                    scale=AA,
                )
            # The bf16 avg copy for chunk j-1 trails by one chunk so the
            # Pool FIFO never stalls the next chunk's fp8 x-cast on it.
            at_tiles.append(at)
            if j > 0:
                nc.gpsimd.tensor_copy(abfs[j - 1][:], at_tiles[j - 1][:])
                nc.scalar.dma_start(avgT[j - 1], abfs[j - 1][:])
        nc.gpsimd.tensor_copy(abfs[NJ - 1][:], at_tiles[NJ - 1][:])
        nc.scalar.dma_start(avgT[NJ - 1], abfs[NJ - 1][:])

        # Phase 2: g^T blocks via fp8 DoubleRow matmul, weight-stationary
        # across the 4 token-chunk PSUM groups. Output blocks run in
        # (input_gate p, forget_gate p) pairs so each chunk's gate combine
        # runs (and its SBUF frees) as early as possible.
        for p in range(NJ - 1):
            if p + 1 < NJ:
                w_tiles[p + 1] = load_w(p + 1)
            wt = w_tiles.pop(p)
            # bf16 sigmoid outputs: the whole gate combine runs as 16-bit
            # DVE ops (2x rate), keeping the combine chain well under the
            # PE's pair cadence even if real DVE rates lag the model.
            st_i = sig_pool.tile([128, L], BF16, name=f"sti{p}", tag="st")
            st_f = sig_pool.tile([128, L], BF16, name=f"stf{p}", tag="st")
            # Halves run sequentially: half 0's group closes mid-pair, so its
            # sigmoid (and the PSUM-recycle it gates) hides completely under
            # half 1's matmul stream.
            for half, st in ((0, st_i), (1, st_f)):
                psb = psum_pool.tile([128, L], FP32, name="psb", tag="psb")
                for j in range(NJ):
                    for tcx in range(NTC):
                        nc.tensor.matmul(
                            psb[:, ts(tcx, TCW)],
                            wt[:, half, j, :, :],
                            cats[j][:, :, ts(tcx, TCW)],
                            start=(j == 0),
                            stop=(j == NJ - 1),
                            perf_mode=mybir.MatmulPerfMode.DoubleRow,
                        )
                ob = p + NJ * half
                nc.scalar.activation(
                    st[:],
                    psb[:],
                    mybir.ActivationFunctionType.Sigmoid,
                    bias=bias_sb[:, ob : ob + 1],
                    scale=SINV,
                )
            # Combine and store (bf16). Pairs 0..NJ-3 run the forget-term
            # multiply on the otherwise-idle Pool engine; the DVE keeps the
            # input-term mul + add full-width (per-op DVE drain overhead is
            # large on HW).
            gtb = gtb_pool.tile([128, L], BF16, name=f"gtb{p}", tag="gtb")
            if p < NJ - 2:
                nc.gpsimd.tensor_mul(st_f[:], st_f[:], abfs[p][:])
            else:
                nc.vector.tensor_mul(st_f[:], st_f[:], abfs[p][:])
            nc.vector.tensor_mul(st_i[:], st_i[:], xts[p][:])
            nc.vector.tensor_add(gtb[:], st_i[:], st_f[:])
            nc.sync.dma_start(gatT[p], gtb[:])

        # Last pair runs token-chunk-outer so each PSUM group closes early
        # and its sigmoid + combine + store pipeline behind the remaining
        # matmuls, shortening the serial tail after the final matmul.
        p = NJ - 1
        wt = w_tiles.pop(p)
        gtb = gtb_pool.tile([128, L], BF16, name="gtb_last", tag="gtb")
        psb_last = {
            half: psum_pool.tile([128, L], FP32, name=f"psbl{half}", tag="psb")
            for half in (0, 1)
        }
        for tcx in range(NTC):
            s = ts(tcx, TCW)
            stp = {}
            for half in (0, 1):
                ps = psb_last[half]
                for jj in range(NJ):
                    nc.tensor.matmul(
                        ps[:, s],
                        wt[:, half, jj, :, :],
                        cats[jj][:, :, s],
                        start=(jj == 0),
                        stop=(jj == NJ - 1),
                        perf_mode=mybir.MatmulPerfMode.DoubleRow,
                    )
                ob = p + NJ * half
                st = stl_pool.tile([128, TCW], BF16, name=f"stl{half}_{tcx}", tag="stl")
                nc.scalar.activation(
                    st[:],
                    ps[:, s],
                    mybir.ActivationFunctionType.Sigmoid,
                    bias=bias_sb[:, ob : ob + 1],
                    scale=SINV,
                )
                stp[half] = st
            nc.vector.tensor_mul(stp[1][:], stp[1][:], abfs[p][:, s])
            nc.vector.tensor_mul(stp[0][:], stp[0][:], xts[p][:, s])
            nc.vector.tensor_add(gtb[:, s], stp[0][:], stp[1][:])
            nc.sync.dma_start(gatT[p][:, s], gtb[:, s])


_CACHE: dict = {}


def build_nc(reps: int | None = None):
    import os as _os

    if reps is None:
        reps = int(_os.environ.get("KREPS", "1"))
    key = ("nc", reps)
    if key not in _CACHE:
        nc = bacc.Bacc(
            "TRN2",
            target_bir_lowering=False,
            debug=False,
            enable_asserts=True,
            num_devices=B,
        )
        with tile.TileContext(nc) as t:
            _tile_body(t, reps=reps)
        nc.compile()
        _CACHE[key] = nc
    return _CACHE[key]


def prep_shared(W_gate: np.ndarray, b_gate: np.ndarray):
    # wdr[p, k, half, j, dr, m] = BW * W_gate[128*(half*NJ+p) + m,
    #                                         dr*1024 + 128*j + k]
    w = (
        (W_gate.astype(np.float32) * BW)
        .reshape(2, NJ, 128, 2, NJ, 128)      # [half, p, m, dr, j, k]
        .transpose(1, 5, 0, 4, 3, 2)          # [p, k, half, j, dr, m]
    )
    wdr = np.ascontiguousarray(w).astype(ml_dtypes.float8_e4m3)
    invd = np.ascontiguousarray(
        np.broadcast_to(
            1.0 / np.arange(1, L + 1, dtype=np.float32)[None, :], (128, L)
        )
    ).astype(ml_dtypes.bfloat16)
    biash = np.ascontiguousarray(b_gate.astype(np.float32).reshape(NOB, 128).T)
    return wdr, invd, biash


def make_in_maps(inputs: np.ndarray, W_gate: np.ndarray, b_gate: np.ndarray):
    wdr, invd, biash = prep_shared(W_gate, b_gate)
    in_maps = []
    for c in range(B):
        # x ships bf16: the matmul path quantizes to fp8 anyway, the cumsum
        # scan keeps fp32 state regardless of input dtype, and the combine's
        # bf16-x error (~0.2%) is far inside the 2e-2 budget. Halves the
        # phase-1 HBM window (8 -> 4 MiB).
        xT_c = (
            np.ascontiguousarray(inputs[c].T)
            .reshape(NJ, 128, L)
            .astype(ml_dtypes.bfloat16)
        )
        in_maps.append({"xT": xT_c, "wdr": wdr, "invd": invd, "biash": biash})
    return in_maps


def kernel(inputs: np.ndarray, W_gate: np.ndarray, b_gate: np.ndarray, **run_kwargs):
    inputs = np.asarray(inputs, dtype=np.float32)
    W_gate = np.asarray(W_gate, dtype=np.float32)
    b_gate = np.asarray(b_gate, dtype=np.float32)
    assert inputs.shape == (B, L, D)

    in_maps = make_in_maps(inputs, W_gate, b_gate)
    nc = build_nc()
    res = bass_utils.run_bass_kernel_spmd(
        nc, in_maps, core_ids=list(range(B)), **run_kwargs
    )

    gating = np.empty((B, L, D), dtype=np.float32)
    average = np.empty((B, L, D), dtype=np.float32)
    for c in range(B):
        gating[c] = (
            res.results[c]["gatT"].astype(np.float32).reshape(D, L).T
        )
        average[c] = (
            res.results[c]["avgT"].astype(np.float32).reshape(D, L).T
        )
    if run_kwargs:
        _CACHE["last_results"] = res
    return gating, average


# revision 21
# speedup vs baseline: 1.3000x; 1.2632x over previous
"""Trainium2 Bass kernel for nn_AverageAttention (B=8, L=2048, D=1024).

Math (per batch b):
    avg[t]  = cumsum(x, axis=t)[t] / (t+1)
    g       = concat([x, avg], -1) @ W_gate.T + b_gate        # (L, 2*D)
    out     = sigmoid(g[:, :D]) * x + sigmoid(g[:, D:]) * avg

Strategy: batch-parallel over 8 NeuronCores (one sequence per core), W_gate
replicated. On-chip layout is transposed (feature-on-partition,
token-on-free) so the cumulative sum is a single DVE tensor_tensor_scan per
128-feature chunk. The gating matmul runs in fp8_e4m3 with
perf_mode=DoubleRow (two 128-row contraction chunks per instruction, ~1.4x
bf16 PE throughput); operands are pre-scaled (x*16, avg*16, W*64) to keep
fp8 out of the subnormal range, and the 1/1024 descale plus bias fold into
the sigmoid PSUM-evacuation on the scalar engine. HBM traffic is kept
minimal: x ships bf16 (halves the phase-1 load window), fp8 weights ride
in (input+forget) pair tiles, invd is bf16, and both outputs are stored
bf16 and upcast on the host (combined rel err ~1.17e-2 vs the 2e-2 gate).
"""

from contextlib import ExitStack

import ml_dtypes
import numpy as np

import concourse.bass as bass
import concourse.bass_utils as bass_utils
import concourse.mybir as mybir
import concourse.tile as tile
from concourse import bacc
from concourse._compat import with_exitstack
from concourse.bass import ts

B, L, D = 8, 2048, 1024
NJ = D // 128        # 8 feature chunks of x / avg (= DoubleRow pairs)
NOB = 2 * D // 128   # 16 output-feature blocks of g
TCW = 512            # matmul moving free-dim (DoubleRow max: 2*512)
NTC = L // TCW       # token chunks

# fp8 pre-scales. S = AX*BW = AA*BW must be uniform across both halves so
# the PSUM accumulation is uniformly scaled; sigmoid evacuation applies 1/S.
AX = 16.0            # x -> fp8 scale
AA = 16.0            # avg -> fp8 scale
BW = 64.0            # W -> fp8 scale
SINV = 1.0 / (AX * BW)

FP32 = mybir.dt.float32
BF16 = mybir.dt.bfloat16
FP8 = mybir.dt.float8e4


@with_exitstack
def _tile_body(ctx: ExitStack, tc: tile.TileContext, reps: int = 1):
    nc = tc.nc

    xT = nc.dram_tensor("xT", (NJ, 128, L), BF16, kind="ExternalInput").ap()
    # wdr[p]: DoubleRow weight tiles for the (input-gate p, forget-gate p)
    # output-block pair, so one DMA fetches both halves of a pair.
    wdr = nc.dram_tensor(
        "wdr", (NJ, 128, 2, NJ, 2, 128), FP8, kind="ExternalInput"
    ).ap()
    invd = nc.dram_tensor("invd", (128, L), BF16, kind="ExternalInput").ap()
    biash = nc.dram_tensor("biash", (128, NOB), FP32, kind="ExternalInput").ap()
    avgT = nc.dram_tensor("avgT", (NJ, 128, L), BF16, kind="ExternalOutput").ap()
    gatT = nc.dram_tensor("gatT", (NJ, 128, L), BF16, kind="ExternalOutput").ap()

    const_pool = ctx.enter_context(tc.tile_pool(name="const", bufs=1))
    x_pool = ctx.enter_context(tc.tile_pool(name="x", bufs=NJ))
    at_pool = ctx.enter_context(tc.tile_pool(name="at", bufs=2))
    abf_pool = ctx.enter_context(tc.tile_pool(name="abf", bufs=NJ))
    cat_pool = ctx.enter_context(tc.tile_pool(name="cat", bufs=NJ))
    w_pool = ctx.enter_context(tc.tile_pool(name="w", bufs=3))
    sig_pool = ctx.enter_context(tc.tile_pool(name="sig", bufs=3))
    stl_pool = ctx.enter_context(tc.tile_pool(name="stl", bufs=4))
    gtb_pool = ctx.enter_context(tc.tile_pool(name="gtb", bufs=2))
    psum_pool = ctx.enter_context(tc.tile_pool(name="psum", bufs=2, space="PSUM"))

    invd_sb = const_pool.tile([128, L], BF16, tag="invd")
    bias_sb = const_pool.tile([128, NOB], FP32, tag="bias")

    for _rep in range(reps):
        # cat[j][:, 0, :] = fp8(AX*x_j); cat[j][:, 1, :] = fp8(AA*avg_j).
        cats = [
            cat_pool.tile([128, 2, L], FP8, tag="cat", name=f"cat{j}")
            for j in range(NJ)
        ]
        xts = [
            x_pool.tile([128, L], BF16, tag="xt", name=f"xt{j}") for j in range(NJ)
        ]
        abfs = [
            abf_pool.tile([128, L], BF16, tag="abf", name=f"abf{j}")
            for j in range(NJ)
        ]

        # DMA head ordering (FIFO per ring): the first weight pair leads the
        # sync ring (the very first matmul needs it and it depends on
        # nothing), then the x chunks stream; constants ride the scalar ring.
        def load_w(p):
            wt = w_pool.tile([128, 2, NJ, 2, 128], FP8, name=f"wt{p}", tag="wt")
            nc.sync.dma_start(wt[:], wdr[p])
            return wt

        w_tiles = {0: load_w(0)}
        if _rep == 0:
            nc.scalar.dma_start(invd_sb[:], invd[:])
            nc.scalar.dma_start(bias_sb[:], biash[:])

        # Phase 1: per chunk j: load x, cast fp8 x (gpsimd), cumsum scan +
        # 1/(t+1) scale (DVE), cast fp8 avg (scalar) and bf16 avg (gpsimd;
        # feeds both the avgT store and the final combine). Chunk 0 runs in
        # two token pieces (piece-chained scan carry) so cat[0] unblocks the
        # PE's first accumulation group earlier.
        PIECES = {0: 2}
        for j in range(NJ):
            np_j = PIECES.get(j, 1)
            pw = L // np_j
            for p in range(np_j):
                nc.sync.dma_start(xts[j][:, ts(p, pw)], xT[j][:, ts(p, pw)])
        at_tiles = []
        for j in range(NJ):
            xt = xts[j]
            ct = sig_pool.tile([128, L], FP32, tag="st", name=f"ct{j}")
            at = at_pool.tile([128, L], FP32)
            np_j = PIECES.get(j, 1)
            pw = L // np_j
            for p in range(np_j):
                s = ts(p, pw)
                nc.gpsimd.tensor_scalar_mul(cats[j][:, 0, s], xt[:, s], AX)
                nc.vector.tensor_tensor_scan(
                    ct[:, s],
                    xt[:, s],
                    xt[:, s],
                    0.0 if p == 0 else ct[:, p * pw - 1 : p * pw],
                    mybir.AluOpType.add,
                    mybir.AluOpType.bypass,
                )
                nc.vector.tensor_mul(at[:, s], ct[:, s], invd_sb[:, s])
                nc.scalar.activation(
                    cats[j][:, 1, s],
                    at[:, s],
                    mybir.ActivationFunctionType.Copy,
                    scale=AA,
                )
            # The bf16 avg copy for chunk j-1 trails by one chunk so the
            # Pool FIFO never stalls the next chunk's fp8 x-cast on it.
            at_tiles.append(at)
            if j > 0:
                nc.gpsimd.tensor_copy(abfs[j - 1][:], at_tiles[j - 1][:])
                nc.scalar.dma_start(avgT[j - 1], abfs[j - 1][:])
        nc.gpsimd.tensor_copy(abfs[NJ - 1][:], at_tiles[NJ - 1][:])
        nc.scalar.dma_start(avgT[NJ - 1], abfs[NJ - 1][:])

        # Phase 2: g^T blocks via fp8 DoubleRow matmul, weight-stationary
        # across the 4 token-chunk PSUM groups. Output blocks run in
        # (input_gate p, forget_gate p) pairs so each chunk's gate combine
        # runs (and its SBUF frees) as early as possible.
        for p in range(NJ - 1):
            if p + 1 < NJ:
                w_tiles[p + 1] = load_w(p + 1)
            wt = w_tiles.pop(p)
            # bf16 sigmoid outputs: the whole gate combine runs as 16-bit
            # DVE ops (2x rate), keeping the combine chain well under the
            # PE's pair cadence even if real DVE rates lag the model.
            st_i = sig_pool.tile([128, L], BF16, name=f"sti{p}", tag="st")
            st_f = sig_pool.tile([128, L], BF16, name=f"stf{p}", tag="st")
            # Halves run sequentially: half 0's group closes mid-pair, so its
            # sigmoid (and the PSUM-recycle it gates) hides completely under
            # half 1's matmul stream.
            for half, st in ((0, st_i), (1, st_f)):
                psb = psum_pool.tile([128, L], FP32, name="psb", tag="psb")
                for j in range(NJ):
                    for tcx in range(NTC):
                        nc.tensor.matmul(
                            psb[:, ts(tcx, TCW)],
                            wt[:, half, j, :, :],
                            cats[j][:, :, ts(tcx, TCW)],
                            start=(j == 0),
                            stop=(j == NJ - 1),
                            perf_mode=mybir.MatmulPerfMode.DoubleRow,
                        )
                ob = p + NJ * half
                nc.scalar.activation(
                    st[:],
                    psb[:],
                    mybir.ActivationFunctionType.Sigmoid,
                    bias=bias_sb[:, ob : ob + 1],
                    scale=SINV,
                )
            # Combine and store (bf16). Pairs 0..NJ-3 run the forget-term
            # multiply on the otherwise-idle Pool engine; the DVE keeps the
            # input-term mul + add full-width (per-op DVE drain overhead is
            # large on HW).
            gtb = gtb_pool.tile([128, L], BF16, name=f"gtb{p}", tag="gtb")
            nc.vector.tensor_mul(st_f[:], st_f[:], abfs[p][:])
            nc.vector.tensor_mul(st_i[:], st_i[:], xts[p][:])
            nc.vector.tensor_add(gtb[:], st_i[:], st_f[:])
            nc.sync.dma_start(gatT[p], gtb[:])

        # Last pair runs token-chunk-outer so each PSUM group closes early
        # and its sigmoid + combine + store pipeline behind the remaining
        # matmuls, shortening the serial tail after the final matmul.
        p = NJ - 1
        wt = w_tiles.pop(p)
        gtb = gtb_pool.tile([128, L], BF16, name="gtb_last", tag="gtb")
        psb_last = {
            half: psum_pool.tile([128, L], FP32, name=f"psbl{half}", tag="psb")
            for half in (0, 1)
        }
        for tcx in range(NTC):
            s = ts(tcx, TCW)
            stp = {}
            for half in (0, 1):
                ps = psb_last[half]
                for jj in range(NJ):
                    nc.tensor.matmul(
                        ps[:, s],
                        wt[:, half, jj, :, :],
                        cats[jj][:, :, s],
                        start=(jj == 0),
                        stop=(jj == NJ - 1),
                        perf_mode=mybir.MatmulPerfMode.DoubleRow,
                    )
                ob = p + NJ * half
                st = stl_pool.tile([128, TCW], BF16, name=f"stl{half}_{tcx}", tag="stl")
                nc.scalar.activation(
                    st[:],
                    ps[:, s],
                    mybir.ActivationFunctionType.Sigmoid,
                    bias=bias_sb[:, ob : ob + 1],
                    scale=SINV,
                )
                stp[half] = st
            nc.vector.tensor_mul(stp[1][:], stp[1][:], abfs[p][:, s])
            nc.vector.tensor_mul(stp[0][:], stp[0][:], xts[p][:, s])
            nc.vector.tensor_add(gtb[:, s], stp[0][:], stp[1][:])
            nc.sync.dma_start(gatT[p][:, s], gtb[:, s])


_CACHE: dict = {}


def build_nc(reps: int | None = None):
    import os as _os

    if reps is None:
        reps = int(_os.environ.get("KREPS", "1"))
    key = ("nc", reps)
    if key not in _CACHE:
        nc = bacc.Bacc(
            "TRN2",
            target_bir_lowering=False,
            debug=False,
            enable_asserts=True,
            num_devices=B,
        )
        with tile.TileContext(nc) as t:
            _tile_body(t, reps=reps)
        nc.compile()
        _CACHE[key] = nc
    return _CACHE[key]


def prep_shared(W_gate: np.ndarray, b_gate: np.ndarray):
    # wdr[p, k, half, j, dr, m] = BW * W_gate[128*(half*NJ+p) + m,
    #                                         dr*1024 + 128*j + k]
    w = (
        (W_gate.astype(np.float32) * BW)
        .reshape(2, NJ, 128, 2, NJ, 128)      # [half, p, m, dr, j, k]
        .transpose(1, 5, 0, 4, 3, 2)          # [p, k, half, j, dr, m]
    )
    wdr = np.ascontiguousarray(w).astype(ml_dtypes.float8_e4m3)
    invd = np.ascontiguousarray(
        np.broadcast_to(
            1.0 / np.arange(1, L + 1, dtype=np.float32)[None, :], (128, L)
        )
    ).astype(ml_dtypes.bfloat16)
    biash = np.ascontiguousarray(b_gate.astype(np.float32).reshape(NOB, 128).T)
    return wdr, invd, biash


def make_in_maps(inputs: np.ndarray, W_gate: np.ndarray, b_gate: np.ndarray):
    wdr, invd, biash = prep_shared(W_gate, b_gate)
    in_maps = []
    for c in range(B):
        # x ships bf16: the matmul path quantizes to fp8 anyway, the cumsum
        # scan keeps fp32 state regardless of input dtype, and the combine's
        # bf16-x error (~0.2%) is far inside the 2e-2 budget. Halves the
        # phase-1 HBM window (8 -> 4 MiB).
        xT_c = (
            np.ascontiguousarray(inputs[c].T)
            .reshape(NJ, 128, L)
            .astype(ml_dtypes.bfloat16)
        )
        in_maps.append({"xT": xT_c, "wdr": wdr, "invd": invd, "biash": biash})
    return in_maps


def kernel(inputs: np.ndarray, W_gate: np.ndarray, b_gate: np.ndarray, **run_kwargs):
    inputs = np.asarray(inputs, dtype=np.float32)
    W_gate = np.asarray(W_gate, dtype=np.float32)
    b_gate = np.asarray(b_gate, dtype=np.float32)
    assert inputs.shape == (B, L, D)

    in_maps = make_in_maps(inputs, W_gate, b_gate)
    nc = build_nc()
    res = bass_utils.run_bass_kernel_spmd(
        nc, in_maps, core_ids=list(range(B)), **run_kwargs
    )

    gating = np.empty((B, L, D), dtype=np.float32)
    average = np.empty((B, L, D), dtype=np.float32)
    for c in range(B):
        gating[c] = (
            res.results[c]["gatT"].astype(np.float32).reshape(D, L).T
        )
        average[c] = (
            res.results[c]["avgT"].astype(np.float32).reshape(D, L).T
        )
    if run_kwargs:
        _CACHE["last_results"] = res
    return gating, average
